# revision 24
# baseline (speedup 1.0000x reference)
"""CenterLoss Trainium2 kernel.

Computes, given features (512, 2048) f32, labels (512,) int, centers
(10000, 2048) f32:
  - center_loss = mean((features - centers[labels])**2)
  - new_centers = sequential per-sample EMA update of centers:
        for i in batch order: c[l_i] <- c[l_i] + 0.5 * (f_i - c[l_i])

The sequential EMA has a closed form per label l with occurrences
i_1 < ... < i_k:
    new_c[l] = 0.5**k * c[l] + sum_j 0.5**(k-j+1) * f[i_j]
so the scan becomes: per-sample weights (host-computed from the labels
alone), a same-label weighted segment-sum U = A @ F (A[i,j] =
[l_i==l_j] * w_j, host-built), a per-label decay, and a scatter of the
final rows  v_l = d_l * centers[l] + U_l  over the bulk copy (duplicate
labels scatter identical rows, so write order is irrelevant).

Sharding: tensor-parallel along feature_dim across 8 cores (256 cols
each). Each core: bulk-copies its (10000, 256) centers slice to the
output (the memory-bound part, ~20.5 MB of HBM r+w), gathers the 512
label rows, computes its slice of the loss + deltas while the copy
streams, then fires one pre-generated scatter-add as soon as the copy
lands. Scalar loss partials are summed on host.
"""

import sys

sys.path.insert(0, "/opt/trn_rl_repo")

import numpy as np

NUM_CLASSES = 10000
FEATURE_DIM = 2048
BATCH = 512
ALPHA = 0.5
NCORES = 8
DS = FEATURE_DIM // NCORES  # 256 per-core feature slice
P = 128  # SBUF partitions
T = BATCH // P  # 4 batch tiles

_cached_nc = None

COPY_DESC_BYTES = 2 ** 16


def _build_program(copy_desc_bytes=None):
    import concourse.bass as bass
    import concourse.tile as tile
    from concourse import bacc, mybir

    nc = bacc.Bacc("TRN2", target_bir_lowering=False, debug=False,
                   num_devices=NCORES)
    f32 = mybir.dt.float32

    centers = nc.dram_tensor("centers", [NUM_CLASSES, DS], f32,
                             kind="ExternalInput").ap()
    feat = nc.dram_tensor("feat", [P, T * DS], f32, kind="ExternalInput").ap()
    at = nc.dram_tensor("at", [P, T * BATCH], mybir.dt.bfloat16,
                        kind="ExternalInput").ap()
    idx = nc.dram_tensor("idx", [P, T], mybir.dt.int32,
                         kind="ExternalInput").ap()
    dec = nc.dram_tensor("dec", [P, T], f32, kind="ExternalInput").ap()
    out_c = nc.dram_tensor("out_centers", [NUM_CLASSES, DS], f32,
                           kind="ExternalOutput").ap()
    loss = nc.dram_tensor("loss_part", [P, 1], f32, kind="ExternalOutput").ap()

    with tile.TileContext(nc) as tc:
        with (
            tc.tile_pool(name="sbuf", bufs=1) as sp,
            tc.tile_pool(name="psum", bufs=1, space="PSUM") as pp,
        ):
            # Index/weight loads via SWDGE (separate SDMA-internal queues
            # from the HWDGE bulk copy, so they are not FIFO'd behind it).
            idx_sb = sp.tile([P, T], mybir.dt.int32)
            nc.gpsimd.dma_start(out=idx_sb[:], in_=idx[:])
            dec_sb = sp.tile([P, T], f32)
            nc.gpsimd.dma_start(out=dec_sb[:], in_=dec[:])

            # Gather G[p, t*DS:(t+1)*DS] = centers[labels[t*P+p]].
            # HW indirect DMA consumes one index per SBUF partition, so one
            # transfer per batch tile of 128 rows.
            g_sb = sp.tile([P, T * DS], f32)
            for t in range(T):
                nc.gpsimd.indirect_dma_start(
                    out=g_sb[:, t * DS:(t + 1) * DS],
                    out_offset=None,
                    in_=centers[:],
                    in_offset=bass.IndirectOffsetOnAxis(
                        ap=idx_sb[:, t:t + 1], axis=0),
                )

            # Bulk data loads on the ACT HWDGE ring (crawls against the
            # copy, but concurrently with the SWDGE gather stream).
            # A is shipped in bf16 (entries are 0 or powers of two — exact)
            # and cast to f32 on DVE.
            f_sb = sp.tile([P, T * DS], f32)
            nc.scalar.dma_start(out=f_sb[:], in_=feat[:])
            at_bf = sp.tile([P, T * BATCH], mybir.dt.bfloat16)
            nc.scalar.dma_start(out=at_bf[:], in_=at[:])
            at_sb = sp.tile([P, T * BATCH], f32)
            nc.vector.tensor_copy(at_sb[:], at_bf[:])

            # U = A @ F (PE), then delta = (d-1) * G + U (DVE), per batch
            # tile. Separate PSUM tags so the groups don't serialize.
            v_sb = sp.tile([P, T * DS], f32)
            for t in range(T):
                u_ps = pp.tile([P, DS], f32, space="PSUM", tag=f"u{t}")
                for k in range(T):
                    nc.tensor.matmul(
                        out=u_ps[:],
                        lhsT=at_sb[:, k * BATCH + t * P:k * BATCH + (t + 1) * P],
                        rhs=f_sb[:, k * DS:(k + 1) * DS],
                        start=(k == 0),
                        stop=(k == T - 1),
                    )
                nc.vector.scalar_tensor_tensor(
                    out=v_sb[:, t * DS:(t + 1) * DS],
                    in0=g_sb[:, t * DS:(t + 1) * DS],
                    scalar=dec_sb[:, t:t + 1],
                    in1=u_ps[:],
                    op0=mybir.AluOpType.mult,
                    op1=mybir.AluOpType.add,
                )

            # Loss slice: sum over (F - G)^2, accumulated per partition.
            # Off the scatter critical path.
            diff = sp.tile([P, T * DS], f32)
            nc.vector.tensor_sub(diff[:], f_sb[:], g_sb[:])
            sq = sp.tile([P, T * DS], f32)
            loss_col = sp.tile([P, 1], f32)
            nc.scalar.activation(
                out=sq[:], in_=diff[:],
                func=mybir.ActivationFunctionType.Square,
                accum_out=loss_col[:],
            )
            nc.scalar.dma_start(out=loss[:], in_=loss_col[:])

            # Bulk copy centers -> out_centers as ONE flat DMA on the
            # otherwise-idle SP HWDGE ring. Emitted late so no compute
            # wait shares a semaphore-lane count with it; it still
            # dispatches immediately (the Sync engine is otherwise empty).
            flat_in = centers.rearrange("a b -> (a b)")
            flat_out = out_c.rearrange("a b -> (a b)")
            nc.sync.dma_start(out=flat_out[:], in_=flat_in[:],
                              max_dma_last_dim=(copy_desc_bytes
                                                or COPY_DESC_BYTES))

            # Scatter the 512 updated rows over the bulk copy. Duplicate
            # labels scatter identical rows, so write order is irrelevant.
            for t in range(T):
                nc.gpsimd.indirect_dma_start(
                    out=out_c[:],
                    out_offset=bass.IndirectOffsetOnAxis(
                        ap=idx_sb[:, t:t + 1], axis=0),
                    in_=v_sb[:, t * DS:(t + 1) * DS],
                    in_offset=None,
                )
    nc.compile()
    return nc


def _get_program():
    global _cached_nc
    if _cached_nc is None:
        _cached_nc = _build_program()
    return _cached_nc


def _host_prep(features, labels, centers):
    """Build per-core input maps."""
    import ml_dtypes

    features = np.ascontiguousarray(np.asarray(features, dtype=np.float32))
    centers = np.asarray(centers, dtype=np.float32)
    labels = np.asarray(labels).astype(np.int64)

    # Per-sample EMA weights from the label sequence alone.
    # occurrence index o_i (0-based) and total count k per label:
    #   w_i = 0.5**(k - o_i), decay = 0.5**k.
    # Only the first occurrence ("representative") of each label carries
    # the update; other occurrences get zero weight rows and zero decay
    # delta so their scatter-add contributes exactly 0.
    counts = {}
    occ = np.empty(BATCH, dtype=np.int64)
    for i, l in enumerate(labels):
        c = counts.get(int(l), 0)
        occ[i] = c
        counts[int(l)] = c + 1
    k = np.array([counts[int(l)] for l in labels], dtype=np.int64)
    w = (0.5 ** (k - occ)).astype(np.float32)
    dec = (0.5 ** k).astype(np.float32)

    # A[i, j] = [l_i == l_j] * w_j; shipped as AT in matmul lhsT layout,
    # bf16 (exact: all entries are 0 or powers of two).
    A = (labels[:, None] == labels[None, :]).astype(np.float32) * w[None, :]
    AT = np.ascontiguousarray(A.T)
    at_sb = np.ascontiguousarray(
        AT.reshape(T, P, BATCH).transpose(1, 0, 2).reshape(P, T * BATCH)
    ).astype(ml_dtypes.bfloat16)

    idx_np = np.ascontiguousarray(labels.reshape(T, P).T).astype(np.int32)
    dec_np = np.ascontiguousarray(dec.reshape(T, P).T).astype(np.float32)

    in_maps = []
    for c in range(NCORES):
        fc = features[:, c * DS:(c + 1) * DS]
        f_sb = np.ascontiguousarray(
            fc.reshape(T, P, DS).transpose(1, 0, 2).reshape(P, T * DS))
        cc = np.ascontiguousarray(centers[:, c * DS:(c + 1) * DS])
        in_maps.append({
            "centers": cc,
            "feat": f_sb,
            "at": at_sb,
            "idx": idx_np,
            "dec": dec_np,
        })
    return in_maps


def run(features, labels, centers, trace=False, **trace_kwargs):
    """Run the device kernel; returns (loss, new_centers, BassKernelResults)."""
    from concourse.bass_utils import run_bass_kernel_spmd

    nc = _get_program()
    in_maps = _host_prep(features, labels, centers)
    res = run_bass_kernel_spmd(nc, in_maps, list(range(NCORES)), trace=trace,
                               **trace_kwargs)
    new_centers = np.concatenate(
        [res.results[c]["out_centers"] for c in range(NCORES)], axis=1)
    sumsq = np.sum([res.results[c]["loss_part"].astype(np.float64).sum()
                    for c in range(NCORES)])
    loss = np.float32(sumsq / (BATCH * FEATURE_DIM))
    return loss, new_centers, res


def kernel(features, labels, centers):
    loss, new_centers, _ = run(features, labels, centers)
    return loss, new_centers


# revision 29
# speedup vs baseline: 1.1023x; 1.1023x over previous
"""CenterLoss Trainium2 kernel.

Computes, given features (512, 2048) f32, labels (512,) int, centers
(10000, 2048) f32:
  - center_loss = mean((features - centers[labels])**2)
  - new_centers = sequential per-sample EMA update of centers:
        for i in batch order: c[l_i] <- c[l_i] + 0.5 * (f_i - c[l_i])

The sequential EMA has a closed form per label l with occurrences
i_1 < ... < i_k:
    new_c[l] = 0.5**k * c[l] + sum_j 0.5**(k-j+1) * f[i_j]
so the scan becomes: per-sample weights (host-computed from the labels
alone), a same-label weighted segment-sum U = A @ F (A[i,j] =
[l_i==l_j] * w_j, host-built), a per-label decay, and a scatter of the
final rows  v_l = d_l * centers[l] + U_l  over the bulk copy (duplicate
labels scatter identical rows, so write order is irrelevant).

Sharding: tensor-parallel along feature_dim across 8 cores (256 cols
each). Each core: bulk-copies its (10000, 256) centers slice to the
output (the memory-bound part, ~20.5 MB of HBM r+w), gathers the 512
label rows, computes its slice of the loss + deltas while the copy
streams, then scatters the updated rows as soon as the copy lands.
Scalar loss partials are summed on host.
"""

import sys

sys.path.insert(0, "/opt/trn_rl_repo")

import numpy as np

NUM_CLASSES = 10000
FEATURE_DIM = 2048
BATCH = 512
ALPHA = 0.5
NCORES = 8
DS = FEATURE_DIM // NCORES  # 256 per-core feature slice
P = 128  # SBUF partitions
T = BATCH // P  # 4 batch tiles

_cached_nc = None

COPY_DESC_BYTES = 2 ** 16


def _build_program(copy_desc_bytes=None, bulk_loads="swdge"):
    import concourse.bass as bass
    import concourse.tile as tile
    from concourse import bacc, mybir

    nc = bacc.Bacc("TRN2", target_bir_lowering=False, debug=False,
                   num_devices=NCORES)
    f32 = mybir.dt.float32

    centers = nc.dram_tensor("centers", [NUM_CLASSES, DS], f32,
                             kind="ExternalInput").ap()
    feat = nc.dram_tensor("feat", [P, T * DS], f32, kind="ExternalInput").ap()
    at = nc.dram_tensor("at", [P, T * BATCH], mybir.dt.bfloat16,
                        kind="ExternalInput").ap()
    idx = nc.dram_tensor("idx", [P, T], mybir.dt.int32,
                         kind="ExternalInput").ap()
    dec = nc.dram_tensor("dec", [P, T], f32, kind="ExternalInput").ap()
    out_c = nc.dram_tensor("out_centers", [NUM_CLASSES, DS], f32,
                           kind="ExternalOutput").ap()
    loss = nc.dram_tensor("loss_part", [P, 1], f32, kind="ExternalOutput").ap()

    with tile.TileContext(nc) as tc:
        with (
            tc.tile_pool(name="sbuf", bufs=1) as sp,
            tc.tile_pool(name="psum", bufs=1, space="PSUM") as pp,
        ):
            # Index/weight loads via SWDGE (separate SDMA-internal queues
            # from the HWDGE bulk copy, so they are not FIFO'd behind it).
            idx_sb = sp.tile([P, T], mybir.dt.int32)
            nc.gpsimd.dma_start(out=idx_sb[:], in_=idx[:])
            dec_sb = sp.tile([P, T], f32)
            nc.gpsimd.dma_start(out=dec_sb[:], in_=dec[:])

            # Gather G[p, t*DS:(t+1)*DS] = centers[labels[t*P+p]].
            # HW indirect DMA consumes one index per SBUF partition, so one
            # transfer per batch tile of 128 rows.
            g_sb = sp.tile([P, T * DS], f32)
            for t in range(T):
                nc.gpsimd.indirect_dma_start(
                    out=g_sb[:, t * DS:(t + 1) * DS],
                    out_offset=None,
                    in_=centers[:],
                    in_offset=bass.IndirectOffsetOnAxis(
                        ap=idx_sb[:, t:t + 1], axis=0),
                )

            # Bulk data loads (default SWDGE; "act" = ACT HWDGE ring).
            # A is shipped in bf16 (entries are 0 or powers of two — exact)
            # and cast to f32 on DVE to halve the bytes crawling against
            # the copy.
            bulk_eng = nc.scalar if bulk_loads == "act" else nc.gpsimd
            f_sb = sp.tile([P, T * DS], f32)
            bulk_eng.dma_start(out=f_sb[:], in_=feat[:])
            at_bf = sp.tile([P, T * BATCH], mybir.dt.bfloat16)
            bulk_eng.dma_start(out=at_bf[:], in_=at[:])
            at_sb = sp.tile([P, T * BATCH], f32)
            nc.vector.tensor_copy(at_sb[:], at_bf[:])

            # U = A @ F (PE), then v = d * G + U (DVE), per batch tile.
            # Separate PSUM tags so the matmul groups don't serialize.
            v_sb = sp.tile([P, T * DS], f32)
            for t in range(T):
                u_ps = pp.tile([P, DS], f32, space="PSUM", tag=f"u{t}")
                for k in range(T):
                    nc.tensor.matmul(
                        out=u_ps[:],
                        lhsT=at_sb[:, k * BATCH + t * P:k * BATCH + (t + 1) * P],
                        rhs=f_sb[:, k * DS:(k + 1) * DS],
                        start=(k == 0),
                        stop=(k == T - 1),
                    )
                nc.vector.scalar_tensor_tensor(
                    out=v_sb[:, t * DS:(t + 1) * DS],
                    in0=g_sb[:, t * DS:(t + 1) * DS],
                    scalar=dec_sb[:, t:t + 1],
                    in1=u_ps[:],
                    op0=mybir.AluOpType.mult,
                    op1=mybir.AluOpType.add,
                )

            # Loss slice: sum over (F - G)^2, accumulated per partition.
            # Off the scatter critical path.
            diff = sp.tile([P, T * DS], f32)
            nc.vector.tensor_sub(diff[:], f_sb[:], g_sb[:])
            sq = sp.tile([P, T * DS], f32)
            loss_col = sp.tile([P, 1], f32)
            nc.scalar.activation(
                out=sq[:], in_=diff[:],
                func=mybir.ActivationFunctionType.Square,
                accum_out=loss_col[:],
            )
            nc.scalar.dma_start(out=loss[:], in_=loss_col[:])

            # Bulk copy centers -> out_centers as ONE flat DMA on the
            # otherwise-idle SP HWDGE ring. Emitted late so no compute
            # wait shares a semaphore-lane count with it; it still
            # dispatches immediately (the Sync engine is otherwise empty).
            flat_in = centers.rearrange("a b -> (a b)")
            flat_out = out_c.rearrange("a b -> (a b)")
            nc.sync.dma_start(out=flat_out[:], in_=flat_in[:],
                              max_dma_last_dim=(copy_desc_bytes
                                                or COPY_DESC_BYTES))

            # Scatter the 512 updated rows over the bulk copy. Duplicate
            # labels scatter identical rows, so write order is irrelevant.
            for t in range(T):
                nc.gpsimd.indirect_dma_start(
                    out=out_c[:],
                    out_offset=bass.IndirectOffsetOnAxis(
                        ap=idx_sb[:, t:t + 1], axis=0),
                    in_=v_sb[:, t * DS:(t + 1) * DS],
                    in_offset=None,
                )
    nc.compile()
    return nc


def _get_program():
    global _cached_nc
    if _cached_nc is None:
        _cached_nc = _build_program()
    return _cached_nc


def _host_prep(features, labels, centers):
    """Build per-core input maps."""
    import ml_dtypes

    features = np.ascontiguousarray(np.asarray(features, dtype=np.float32))
    centers = np.asarray(centers, dtype=np.float32)
    labels = np.asarray(labels).astype(np.int64)

    # Per-sample EMA weights from the label sequence alone.
    # occurrence index o_i (0-based) and total count k per label:
    #   w_i = 0.5**(k - o_i), decay = 0.5**k.
    counts = {}
    occ = np.empty(BATCH, dtype=np.int64)
    for i, l in enumerate(labels):
        c = counts.get(int(l), 0)
        occ[i] = c
        counts[int(l)] = c + 1
    k = np.array([counts[int(l)] for l in labels], dtype=np.int64)
    w = (0.5 ** (k - occ)).astype(np.float32)
    dec = (0.5 ** k).astype(np.float32)

    # A[i, j] = [l_i == l_j] * w_j; shipped as AT in matmul lhsT layout,
    # bf16 (exact: all entries are 0 or powers of two).
    A = (labels[:, None] == labels[None, :]).astype(np.float32) * w[None, :]
    AT = np.ascontiguousarray(A.T)
    at_sb = np.ascontiguousarray(
        AT.reshape(T, P, BATCH).transpose(1, 0, 2).reshape(P, T * BATCH)
    ).astype(ml_dtypes.bfloat16)

    idx_np = np.ascontiguousarray(labels.reshape(T, P).T).astype(np.int32)
    dec_np = np.ascontiguousarray(dec.reshape(T, P).T).astype(np.float32)

    in_maps = []
    for c in range(NCORES):
        fc = features[:, c * DS:(c + 1) * DS]
        f_sb = np.ascontiguousarray(
            fc.reshape(T, P, DS).transpose(1, 0, 2).reshape(P, T * DS))
        cc = np.ascontiguousarray(centers[:, c * DS:(c + 1) * DS])
        in_maps.append({
            "centers": cc,
            "feat": f_sb,
            "at": at_sb,
            "idx": idx_np,
            "dec": dec_np,
        })
    return in_maps


def run(features, labels, centers, trace=False, **trace_kwargs):
    """Run the device kernel; returns (loss, new_centers, BassKernelResults)."""
    from concourse.bass_utils import run_bass_kernel_spmd

    nc = _get_program()
    in_maps = _host_prep(features, labels, centers)
    res = run_bass_kernel_spmd(nc, in_maps, list(range(NCORES)), trace=trace,
                               **trace_kwargs)
    new_centers = np.concatenate(
        [res.results[c]["out_centers"] for c in range(NCORES)], axis=1)
    sumsq = np.sum([res.results[c]["loss_part"].astype(np.float64).sum()
                    for c in range(NCORES)])
    loss = np.float32(sumsq / (BATCH * FEATURE_DIM))
    return loss, new_centers, res


def kernel(features, labels, centers):
    loss, new_centers, _ = run(features, labels, centers)
    return loss, new_centers
